# revision 16
# baseline (speedup 1.0000x reference)
"""Causal self-attention (B=2, T=4096, C=768, H=12) on 8 TRN2 NeuronCores.

Sharding: batch x head-group. Core c handles batch b=c//4 and heads
h0..h0+2 where h0 = 3*(c%4). Each core computes qkv projection for its 3
heads, full causal attention, and a partial output projection; the host
sums the 4 partials per batch and adds the projection bias.

On-chip layout is feature-major ("transposed"): qT/kT [D, T] feed the
scores matmul directly, scores^T [k, q] feeds att@v with v in natural
layout, and the attention output stays transposed to feed the output
projection as the stationary operand (producing natural-layout y).
Matmuls run in float32r (~tf32). The softmax denominator comes free as a
65th "ones" column of v; normalization uses reciprocal_approx_fast + a
gpsimd partition broadcast.
"""

import sys

for _p in ("/opt/trn_rl_repo",):
    if _p not in sys.path:
        sys.path.insert(0, _p)

from contextlib import ExitStack

import numpy as np

import concourse.bass as bass  # noqa: F401  (engine classes referenced via nc)
import concourse.mybir as mybir
import concourse.tile as tile
from concourse import bacc
from concourse.bass_utils import run_bass_kernel_spmd
from concourse.masks import make_identity
from concourse.tile_rust import add_dep_helper

f32 = mybir.dt.float32
f32r = mybir.dt.float32r
AF = mybir.ActivationFunctionType

C = 768
D = 64
N_HEAD = 12
HPC = 3  # heads per core
N_CORES = 8

# wq column slots: q01 | k01 | v01 | (q2 stacked over k2) | v2
SLOTS = [(0, 128), (128, 256), (256, 384), (384, 512), (512, 576)]


def build_nc(T):
    NT = T // 512  # q tiles
    KT = T // 128  # k tiles
    CK = C // 128  # contraction chunks for qkv

    nc = bacc.Bacc("TRN2", target_bir_lowering=False, debug=False,
                   num_devices=N_CORES)
    xt_d = nc.dram_tensor("xt", [C, T], f32r, kind="ExternalInput").ap()
    wq_d = nc.dram_tensor("wq", [C, 576], f32r, kind="ExternalInput").ap()
    bq_d = nc.dram_tensor("bq", [128, 5], f32, kind="ExternalInput").ap()
    wp_d = nc.dram_tensor("wp", [HPC * D, C], f32r, kind="ExternalInput").ap()
    y_d = nc.dram_tensor("y", [T, C], f32, kind="ExternalOutput").ap()
    import os
    dbg = os.environ.get("KDBG") == "1"
    # internal DRAM scratch for the softmax-reciprocal row broadcast
    rsc_d = nc.dram_tensor("rscratch", [NT * HPC, 512], f32,
                           **({"kind": "ExternalOutput"} if dbg else {})).ap()
    dbg_out = {}
    if dbg:
        for nm, shp in [("d_qAB", [128, T]), ("d_kAB", [128, T]),
                        ("d_qC", [128, T]), ("d_kC", [128, T]),
                        ("d_vaug", [128, KT * 195]),
                        ("d_ao0", [64, T]), ("d_ao1", [64, T]),
                        ("d_ao2", [64, T]), ("d_bc", [64, 512]),
                        ("d_eb", [128, 3072]), ("d_attv", [65, 512])]:
            dbg_out[nm] = nc.dram_tensor(nm, shp, f32, kind="ExternalOutput").ap()

    with tile.TileContext(nc) as tc, ExitStack() as ctx:
        sb = ctx.enter_context(tc.tile_pool(name="sb", bufs=1))

        # persistent tensors (live for the whole kernel)
        bq_sb = sb.tile([128, 5], f32, tag="bq")
        qT_AB = sb.tile([128, T], f32r, tag="qAB")
        kT_AB = sb.tile([128, T], f32r, tag="kAB")
        qT_C = sb.tile([128, T], f32r, tag="qC")
        kT_C = sb.tile([128, T], f32r, tag="kC")
        ident = sb.tile([128, 128], f32, tag="ident")
        ones_f = sb.tile([128, 1], f32, tag="ones")

        nc.sync.dma_start(bq_sb[:], bq_d)
        make_identity(nc, ident[:])
        nc.vector.memset(ones_f[:], 1.0)
        # causal masks for the 4 diagonal-band positions: keep col-p >= 128*r
        cmask = sb.tile([128, 4 * 512], f32, tag="cmask")
        nc.gpsimd.memset(cmask[:], 1.0)
        for r in range(4):
            nc.gpsimd.affine_select(
                cmask[:, r * 512:(r + 1) * 512], cmask[:, r * 512:(r + 1) * 512],
                pattern=[[1, 512]], compare_op=mybir.AluOpType.is_ge, fill=0.0,
                base=-128 * r, channel_multiplier=-1)

        # vaug lives phases 2-3; vpool (inside it) only phases 1-2
        vaugp = ctx.enter_context(tc.tile_pool(name="vaugp", bufs=1))
        es_v = ExitStack()
        vp = es_v.enter_context(tc.tile_pool(name="vpool", bufs=1))
        vT01 = vp.tile([128, T], f32, tag="v01")
        vT2 = vp.tile([64, T], f32, tag="v2")

        # ---------------- phase 1: qkv projection (transposed) --------------
        with tc.tile_pool(name="wqp", bufs=1) as wqp, \
             tc.tile_pool(name="xtp", bufs=3) as xt_pool, \
             tc.tile_pool(name="qkvps", bufs=1, space="PSUM") as qkv_ps:
            wq_sb = [wqp.tile([128, 576], f32r, tag=f"wq{c}", name=f"wq{c}")
                     for c in range(CK)]
            for c in range(CK):
                nc.sync.dma_start(wq_sb[c][:], wq_d[c * 128:(c + 1) * 128, :])
            for j in range(NT):
                jsl = bass.ts(j, 512)
                ps = [qkv_ps.tile([128, 512], f32, tag=f"s{k}", name=f"ps{k}") for k in range(4)]
                ps.append(qkv_ps.tile([64, 512], f32, tag="s4", name="ps4"))
                for c in range(CK):
                    xt_t = xt_pool.tile([128, 512], f32r, tag="xt")
                    nc.sync.dma_start(
                        xt_t[:], xt_d[c * 128:(c + 1) * 128, j * 512:(j + 1) * 512])
                    for s, (c0, c1) in enumerate(SLOTS):
                        nc.tensor.matmul(ps[s][:], wq_sb[c][:, c0:c1], xt_t[:],
                                         start=(c == 0), stop=(c == CK - 1))
                nc.vector.tensor_scalar_add(qT_AB[:, jsl], ps[0][:], bq_sb[:, 0:1])
                nc.vector.tensor_scalar_add(kT_AB[:, jsl], ps[1][:], bq_sb[:, 1:2])
                nc.vector.tensor_scalar_add(vT01[:, jsl], ps[2][:], bq_sb[:, 2:3])
                nc.vector.tensor_scalar_add(qT_C[0:64, jsl], ps[3][0:64, :],
                                            bq_sb[0:64, 3:4])
                nc.vector.tensor_scalar_add(kT_C[64:128, jsl], ps[3][64:128, :],
                                            bq_sb[64:128, 3:4])
                nc.vector.tensor_scalar_add(vT2[:, jsl], ps[4][:], bq_sb[0:64, 4:5])
            # duplicate head-2 q/k into the other 64-partition strip
            nc.sync.dma_start(qT_C[64:128, :], qT_C[0:64, :])
            nc.sync.dma_start(kT_C[0:64, :], kT_C[64:128, :])
            if dbg:
                nc.sync.dma_start(dbg_out["d_qAB"], qT_AB[:].bitcast(f32))
                nc.sync.dma_start(dbg_out["d_kAB"], kT_AB[:].bitcast(f32))
                nc.sync.dma_start(dbg_out["d_qC"], qT_C[:].bitcast(f32))
                nc.sync.dma_start(dbg_out["d_kC"], kT_C[:].bitcast(f32))

        # ---------------- phase 2: v -> natural layout + ones column --------
        if True:
          v_aug = vaugp.tile([128, KT * 195], f32r, tag="vaug")
          with tc.tile_pool(name="tps", bufs=3, space="PSUM") as tp_ps:
            for ki in range(KT):
                ksl = bass.ts(ki, 128)
                base = ki * 195
                p01 = tp_ps.tile([128, 128], f32, tag="tp01")
                nc.tensor.transpose(p01[:], vT01[:, ksl], ident[:])
                p2t = tp_ps.tile([128, 64], f32, tag="tp2")
                nc.tensor.transpose(p2t[:], vT2[:, ksl], ident[0:64, 0:64])
                nc.vector.tensor_copy(v_aug[:, base:base + 64], p01[:, 0:64])
                nc.vector.tensor_copy(v_aug[:, base + 65:base + 129], p01[:, 64:128])
                nc.vector.tensor_copy(v_aug[:, base + 130:base + 194], p2t[:])
            ones_cols = v_aug[:].rearrange("p (k c) -> p k c", c=65)[:, :, 64:65]
            nc.vector.tensor_copy(
                ones_cols, ones_f[:, 0:1, None].broadcast_to([128, 3 * KT, 1]))
          if dbg:
              nc.sync.dma_start(dbg_out["d_vaug"], v_aug[:].bitcast(f32))
          es_v.close()  # vT buffers no longer needed

          # ---------------- phase 3: attention -------------------------------
          aop = ctx.enter_context(tc.tile_pool(name="aop", bufs=1))
          aoT = [aop.tile([64, T], f32r, tag=f"aoT{h}", name=f"aoT{h}")
                 for h in range(HPC)]
          with tc.tile_pool(name="scps", bufs=1, space="PSUM") as sc_ps, \
             tc.tile_pool(name="avps", bufs=2, space="PSUM") as av_ps, \
             tc.tile_pool(name="ebp", bufs=2) as eb_pool, \
             tc.tile_pool(name="nrm", bufs=2) as nrm:
            for j in range(NT):
                jsl = bass.ts(j, 512)
                nk = 4 * j + 4
                for slot in ("AB", "C"):
                    if slot == "AB":
                        heads = [0, 1]
                        group = 3  # k-tiles per round (2 banks each)
                    else:
                        heads = [2]
                        group = 6
                    att = {h: av_ps.tile([65, 512], f32, tag="attv", name=f"attv{h}")
                           for h in heads}
                    for g0 in range(0, nk, group):
                        ks = list(range(g0, min(g0 + group, nk)))
                        nbank = len(ks) * len(heads)
                        pr = sc_ps.tile([128, 3072], f32, tag="sc")
                        banks = []  # (bank, ki, head)
                        for idx, ki in enumerate(ks):
                            ksl = bass.ts(ki, 128)
                            if slot == "AB":
                                for hh in (0, 1):
                                    b = idx * 2 + hh
                                    r0, r1 = 64 * hh, 64 * hh + 64
                                    nc.tensor.matmul(
                                        pr[:, bass.ts(b, 512)],
                                        kT_AB[r0:r1, ksl], qT_AB[r0:r1, jsl],
                                        start=True, stop=True)
                                    banks.append((b, ki, hh))
                            else:
                                strip = idx % 2
                                r0, r1 = 64 * strip, 64 * strip + 64
                                nc.tensor.matmul(
                                    pr[:, bass.ts(idx, 512)],
                                    kT_C[r0:r1, ksl], qT_C[r0:r1, jsl],
                                    start=True, stop=True)
                                banks.append((idx, ki, 2))
                        eb = eb_pool.tile([128, 3072], f32r, tag="eb")
                        nc.scalar.activation(eb[:, 0:nbank * 512],
                                             pr[:, 0:nbank * 512],
                                             AF.Exp, scale=0.125)
                        if dbg and j == NT - 1 and slot == "C" and g0 == 0:
                            nc.sync.dma_start(dbg_out["d_eb"][:, 0:nbank * 512],
                                              eb[:, 0:nbank * 512].bitcast(f32))
                        for b, ki, h in banks:
                            if ki >= 4 * j:  # diagonal band: causal mask
                                bsl = bass.ts(b, 512)
                                r = ki - 4 * j
                                nc.vector.tensor_mul(
                                    eb[:, bsl], eb[:, bsl],
                                    cmask[:, bass.ts(r, 512)])
                        for b, ki, h in banks:
                            nc.tensor.matmul(
                                att[h][:], v_aug[:, ki * 195 + 65 * h:
                                                 ki * 195 + 65 * h + 65],
                                eb[:, bass.ts(b, 512)],
                                start=(ki == 0), stop=(ki == nk - 1),
                                skip_group_check=True)
                    for h in heads:
                        if dbg and j == NT - 1 and h == 2:
                            datt = nrm.tile([65, 512], f32, tag="datt")
                            nc.vector.tensor_copy(datt[:], att[h][:])
                            nc.sync.dma_start(dbg_out["d_attv"], datt[:])
                        # denominator row (psum p64) -> sbuf, then broadcast
                        # across 64 partitions via a DRAM round-trip (stride-0
                        # leading dim is DRAM-only). Tile does not dep-track
                        # DRAM, so wire the RAW edge explicitly. The recip runs
                        # after the broadcast: custom-dve ops misbehave at
                        # nonzero base partitions.
                        scrA = nrm.tile([65, 512], f32, tag="scrA")
                        nc.vector.tensor_copy(scrA[64:65, :], att[h][64:65, :])
                        row_d = rsc_d[j * HPC + h, :]
                        wr = nc.sync.dma_start(row_d[None, :], scrA[64:65, :])
                        bc = nrm.tile([64, 512], f32, tag="bc")
                        rd = nc.gpsimd.dma_start(
                            out=bc[:], in_=bass.AP(row_d.tensor, row_d.offset,
                                                   [[0, 64], [1, 512]]))
                        add_dep_helper(rd.ins, wr.ins,
                                       reason="rscratch RAW (dram roundtrip)")
                        rcp = nrm.tile([64, 512], f32, tag="rcp")
                        nc.vector.reciprocal_approx_fast(out=rcp[:], in_=bc[:])
                        nc.vector.tensor_mul(aoT[h][:, jsl], att[h][0:64, :], rcp[:])
                        if dbg and j == NT - 1 and h == 2:
                            nc.sync.dma_start(dbg_out["d_bc"], bc[:])

        if dbg:
            for h in range(HPC):
                nc.sync.dma_start(dbg_out[f"d_ao{h}"], aoT[h][:].bitcast(f32))

        # ---------------- phase 4: output projection -------------------------
        with tc.tile_pool(name="pps", bufs=2, space="PSUM") as pr_ps, \
             tc.tile_pool(name="wpp", bufs=1) as wpp, \
             tc.tile_pool(name="yp", bufs=3) as y_pool:
            wp_sb = [wpp.tile([64, C], f32r, tag=f"wp{h}", name=f"wp{h}")
                     for h in range(HPC)]
            for h in range(HPC):
                nc.sync.dma_start(wp_sb[h][:], wp_d[h * 64:(h + 1) * 64, :])
            for m in range(T // 128):
                msl = bass.ts(m, 128)
                y_sb = y_pool.tile([128, C], f32, tag="y")
                for ns in range(2):
                    py = pr_ps.tile([128, 384], f32, tag=f"py{ns}")
                    for h in range(HPC):
                        nc.tensor.matmul(py[:], aoT[h][:, msl],
                                         wp_sb[h][:, ns * 384:(ns + 1) * 384],
                                         start=(h == 0), stop=(h == HPC - 1))
                    nc.vector.tensor_copy(y_sb[:, ns * 384:(ns + 1) * 384], py[:])
                nc.sync.dma_start(y_d[m * 128:(m + 1) * 128, :], y_sb[:])

    nc.compile()
    return nc


_NC_CACHE = {}


def _get_nc(T):
    if T not in _NC_CACHE:
        _NC_CACHE[T] = build_nc(T)
    return _NC_CACHE[T]


def make_core_inputs(x, W_attn, b_attn, W_proj):
    """Host-side prep: per-core input dicts (see module docstring)."""
    B, T, _ = x.shape
    xts = [np.ascontiguousarray(x[b].T) for b in range(B)]
    in_maps = []
    for core in range(N_CORES):
        b = core // (N_CORES // B)
        h0 = HPC * (core % (N_CORES // B))
        ccols = slice(h0 * D, (h0 + 2) * D)      # first two heads
        c2 = slice((h0 + 2) * D, (h0 + 3) * D)   # third head
        # reference splits qkv as (k, q, v): k cols 0:C, q cols C:2C, v 2C:3C
        q01 = W_attn[:, C:2 * C][:, ccols]
        k01 = W_attn[:, 0:C][:, ccols]
        v01 = W_attn[:, 2 * C:3 * C][:, ccols]
        q2 = W_attn[:, C:2 * C][:, c2]
        k2 = W_attn[:, 0:C][:, c2]
        v2 = W_attn[:, 2 * C:3 * C][:, c2]
        wq = np.ascontiguousarray(
            np.concatenate([q01, k01, v01, q2, k2, v2], axis=1))
        bq = np.zeros((128, 5), np.float32)
        bq[:, 0] = b_attn[C:2 * C][ccols]
        bq[:, 1] = b_attn[0:C][ccols]
        bq[:, 2] = b_attn[2 * C:3 * C][ccols]
        bq[0:64, 3] = b_attn[C:2 * C][c2]
        bq[64:128, 3] = b_attn[0:C][c2]
        bq[0:64, 4] = b_attn[2 * C:3 * C][c2]
        wp = np.ascontiguousarray(W_proj[h0 * D:(h0 + HPC) * D, :])
        in_maps.append({"xt": xts[b], "wq": wq, "bq": bq, "wp": wp})
    return in_maps


def kernel(x, W_attn, b_attn, W_proj, b_proj):
    x = np.asarray(x, dtype=np.float32)
    W_attn = np.asarray(W_attn, dtype=np.float32)
    b_attn = np.asarray(b_attn, dtype=np.float32)
    W_proj = np.asarray(W_proj, dtype=np.float32)
    b_proj = np.asarray(b_proj, dtype=np.float32)
    B, T, _ = x.shape

    nc = _get_nc(T)
    in_maps = make_core_inputs(x, W_attn, b_attn, W_proj)
    res = run_bass_kernel_spmd(nc, in_maps, list(range(N_CORES)))
    global LAST_RUN
    LAST_RUN = res

    gpb = N_CORES // B
    out = np.empty((B, T, C), np.float32)
    for b in range(B):
        acc = res.results[b * gpb]["y"].astype(np.float32)
        for g in range(1, gpb):
            acc = acc + res.results[b * gpb + g]["y"]
        out[b] = acc + b_proj[None, :]
    return out


# revision 21
# speedup vs baseline: 1.3212x; 1.3212x over previous
"""Causal self-attention (B=2, T=4096, C=768, H=12) on 8 TRN2 NeuronCores.

Sharding: batch x head-group. Core c handles batch b=c//4 and heads
h0..h0+2 where h0 = 3*(c%4). Each core computes qkv projection for its 3
heads, full causal attention, and a partial output projection; the host
sums the 4 partials per batch and adds the projection bias.

On-chip layout is feature-major ("transposed"): qT/kT [D, T] feed the
scores matmul directly, scores^T [k, q] feeds att@v with v in natural
layout, and the attention output stays transposed to feed the output
projection as the stationary operand (producing natural-layout y).
Matmuls run in float32r (~tf32). The softmax denominator comes free as a
65th "ones" column of v; normalization uses reciprocal_approx_fast + a
gpsimd partition broadcast.
"""

import sys

for _p in ("/opt/trn_rl_repo",):
    if _p not in sys.path:
        sys.path.insert(0, _p)

from contextlib import ExitStack

import numpy as np

import concourse.bass as bass  # noqa: F401  (engine classes referenced via nc)
import concourse.mybir as mybir
import concourse.tile as tile
from concourse import bacc
from concourse.bass_utils import run_bass_kernel_spmd
from concourse.masks import make_identity
from concourse.tile_rust import add_dep_helper

f32 = mybir.dt.float32
f32r = mybir.dt.float32r
AF = mybir.ActivationFunctionType

C = 768
D = 64
N_HEAD = 12
HPC = 3  # heads per core
N_CORES = 8

# wq column slots: q01 | k01 | v01 | (q2 stacked over k2) | v2
SLOTS = [(0, 128), (128, 256), (256, 384), (384, 512), (512, 576)]


def build_nc(T):
    NT = T // 512  # q tiles
    KT = T // 128  # k tiles
    CK = C // 128  # contraction chunks for qkv

    nc = bacc.Bacc("TRN2", target_bir_lowering=False, debug=False,
                   num_devices=N_CORES)
    xt_d = nc.dram_tensor("xt", [C, T], f32r, kind="ExternalInput").ap()
    wq_d = nc.dram_tensor("wq", [C, 576], f32r, kind="ExternalInput").ap()
    bq_d = nc.dram_tensor("bq", [128, 5], f32, kind="ExternalInput").ap()
    wp_d = nc.dram_tensor("wp", [HPC * D, C], f32r, kind="ExternalInput").ap()
    y_d = nc.dram_tensor("y", [T, C], f32, kind="ExternalOutput").ap()
    import os
    dbg = os.environ.get("KDBG") == "1"
    kphase = int(os.environ.get("KPHASE", "4"))
    # internal DRAM scratch for the softmax-reciprocal row broadcast
    rsc_d = nc.dram_tensor("rscratch", [NT * HPC, 512], f32,
                           **({"kind": "ExternalOutput"} if dbg else {})).ap()
    dbg_out = {}
    if dbg:
        for nm, shp in [("d_qAB", [128, T]), ("d_kAB", [128, T]),
                        ("d_qC", [128, T]), ("d_kC", [128, T]),
                        ("d_vaug", [128, KT * 195]),
                        ("d_ao0", [64, T]), ("d_ao1", [64, T]),
                        ("d_ao2", [64, T]), ("d_bc", [64, 512]),
                        ("d_eb", [128, 3072]), ("d_attv", [65, 512])]:
            dbg_out[nm] = nc.dram_tensor(nm, shp, f32, kind="ExternalOutput").ap()

    with tile.TileContext(nc) as tc, ExitStack() as ctx:
        sb = ctx.enter_context(tc.tile_pool(name="sb", bufs=1))

        # persistent tensors (live for the whole kernel)
        bq_sb = sb.tile([128, 5], f32, tag="bq")
        qT_AB = sb.tile([128, T], f32r, tag="qAB")
        kT_AB = sb.tile([128, T], f32r, tag="kAB")
        qT_C = sb.tile([128, T], f32r, tag="qC")
        kT_C = sb.tile([128, T], f32r, tag="kC")
        ident = sb.tile([128, 128], f32, tag="ident")
        ones_f = sb.tile([128, 1], f32, tag="ones")

        nc.sync.dma_start(bq_sb[:], bq_d)
        make_identity(nc, ident[:])
        nc.vector.memset(ones_f[:], 1.0)
        # causal masks for the 4 diagonal-band positions: keep col-p >= 128*r
        cmask = sb.tile([128, 4 * 512], f32, tag="cmask")
        nc.gpsimd.memset(cmask[:], 1.0)
        for r in range(4):
            nc.gpsimd.affine_select(
                cmask[:, r * 512:(r + 1) * 512], cmask[:, r * 512:(r + 1) * 512],
                pattern=[[1, 512]], compare_op=mybir.AluOpType.is_ge, fill=0.0,
                base=-128 * r, channel_multiplier=-1)

        # vaug lives phases 2-3; vpool (inside it) only phases 1-2
        vaugp = ctx.enter_context(tc.tile_pool(name="vaugp", bufs=1))
        es_v = ExitStack()
        vp = es_v.enter_context(tc.tile_pool(name="vpool", bufs=1))
        vT01 = vp.tile([128, T], f32, tag="v01")
        vT2 = vp.tile([64, T], f32, tag="v2")

        # ---------------- phase 1: qkv projection (transposed) --------------
        with tc.tile_pool(name="wqp", bufs=1) as wqp, \
             tc.tile_pool(name="xtp", bufs=3) as xt_pool, \
             tc.tile_pool(name="qkvps", bufs=1, space="PSUM") as qkv_ps:
            wq_sb = [wqp.tile([128, 576], f32r, tag=f"wq{c}", name=f"wq{c}")
                     for c in range(CK)]
            for c in range(CK):
                nc.sync.dma_start(wq_sb[c][:], wq_d[c * 128:(c + 1) * 128, :])
            for j in range(NT):
                jsl = bass.ts(j, 512)
                ps = [qkv_ps.tile([128, 512], f32, tag=f"s{k}", name=f"ps{k}") for k in range(4)]
                ps.append(qkv_ps.tile([64, 512], f32, tag="s4", name="ps4"))
                for c in range(CK):
                    xt_t = xt_pool.tile([128, 512], f32r, tag="xt")
                    nc.sync.dma_start(
                        xt_t[:], xt_d[c * 128:(c + 1) * 128, j * 512:(j + 1) * 512])
                    for s, (c0, c1) in enumerate(SLOTS):
                        nc.tensor.matmul(ps[s][:], wq_sb[c][:, c0:c1], xt_t[:],
                                         start=(c == 0), stop=(c == CK - 1))
                nc.vector.tensor_scalar_add(qT_AB[:, jsl], ps[0][:], bq_sb[:, 0:1])
                nc.vector.tensor_scalar_add(kT_AB[:, jsl], ps[1][:], bq_sb[:, 1:2])
                nc.vector.tensor_scalar_add(vT01[:, jsl], ps[2][:], bq_sb[:, 2:3])
                nc.vector.tensor_scalar_add(qT_C[0:64, jsl], ps[3][0:64, :],
                                            bq_sb[0:64, 3:4])
                nc.vector.tensor_scalar_add(kT_C[64:128, jsl], ps[3][64:128, :],
                                            bq_sb[64:128, 3:4])
                nc.vector.tensor_scalar_add(vT2[:, jsl], ps[4][:], bq_sb[0:64, 4:5])
            # duplicate head-2 q/k into the other 64-partition strip
            nc.sync.dma_start(qT_C[64:128, :], qT_C[0:64, :])
            nc.sync.dma_start(kT_C[0:64, :], kT_C[64:128, :])
            if dbg:
                nc.sync.dma_start(dbg_out["d_qAB"], qT_AB[:].bitcast(f32))
                nc.sync.dma_start(dbg_out["d_kAB"], kT_AB[:].bitcast(f32))
                nc.sync.dma_start(dbg_out["d_qC"], qT_C[:].bitcast(f32))
                nc.sync.dma_start(dbg_out["d_kC"], kT_C[:].bitcast(f32))

        # ---------------- phase 2: v -> natural layout + ones column --------
        if kphase >= 2:
          v_aug = vaugp.tile([128, KT * 195], f32r, tag="vaug")
          with tc.tile_pool(name="tps", bufs=3, space="PSUM") as tp_ps:
            for ki in range(KT):
                ksl = bass.ts(ki, 128)
                base = ki * 195
                p01 = tp_ps.tile([128, 128], f32, tag="tp01")
                nc.tensor.transpose(p01[:], vT01[:, ksl], ident[:])
                p2t = tp_ps.tile([128, 64], f32, tag="tp2")
                nc.tensor.transpose(p2t[:], vT2[:, ksl], ident[0:64, 0:64])
                nc.vector.tensor_copy(v_aug[:, base:base + 64], p01[:, 0:64])
                nc.vector.tensor_copy(v_aug[:, base + 65:base + 129], p01[:, 64:128])
                nc.vector.tensor_copy(v_aug[:, base + 130:base + 194], p2t[:])
            ones_cols = v_aug[:].rearrange("p (k c) -> p k c", c=65)[:, :, 64:65]
            nc.vector.tensor_copy(
                ones_cols, ones_f[:, 0:1, None].broadcast_to([128, 3 * KT, 1]))
          if dbg:
              nc.sync.dma_start(dbg_out["d_vaug"], v_aug[:].bitcast(f32))
          es_v.close()  # vT buffers no longer needed

          # ---------------- phase 3: attention -------------------------------
          aop = ctx.enter_context(tc.tile_pool(name="aop", bufs=1))
          aoT = [aop.tile([64, T], f32r, tag=f"aoT{h}", name=f"aoT{h}")
                 for h in range(HPC)]
          with tc.tile_pool(name="scps", bufs=2, space="PSUM") as sc_ps, \
             tc.tile_pool(name="avps", bufs=3, space="PSUM") as av_ps, \
             tc.tile_pool(name="ebp", bufs=3) as eb_pool, \
             tc.tile_pool(name="nrm", bufs=3) as nrm:
            for j in range(NT if kphase >= 3 else 0):
                jsl = bass.ts(j, 512)
                nk = 4 * j + 4
                for slot in ("AB", "C"):
                    if slot == "AB":
                        heads = [0, 1]
                        group = 1  # k-tiles per round (2 banks each)
                    else:
                        heads = [2]
                        group = 2
                    att = {h: av_ps.tile([65, 512], f32, tag="attv", name=f"attv{h}")
                           for h in heads}
                    for g0 in range(0, nk, group):
                        ks = list(range(g0, min(g0 + group, nk)))
                        nbank = len(ks) * len(heads)
                        pr = sc_ps.tile([128, 1024], f32, tag="sc")
                        banks = []  # (bank, ki, head)
                        for idx, ki in enumerate(ks):
                            ksl = bass.ts(ki, 128)
                            if slot == "AB":
                                for hh in (0, 1):
                                    b = idx * 2 + hh
                                    r0, r1 = 64 * hh, 64 * hh + 64
                                    nc.tensor.matmul(
                                        pr[:, bass.ts(b, 512)],
                                        kT_AB[r0:r1, ksl], qT_AB[r0:r1, jsl],
                                        start=True, stop=True)
                                    banks.append((b, ki, hh))
                            else:
                                strip = idx % 2
                                r0, r1 = 64 * strip, 64 * strip + 64
                                nc.tensor.matmul(
                                    pr[:, bass.ts(idx, 512)],
                                    kT_C[r0:r1, ksl], qT_C[r0:r1, jsl],
                                    start=True, stop=True)
                                banks.append((idx, ki, 2))
                        eb = eb_pool.tile([128, 1024], f32r, tag="eb")
                        nc.scalar.activation(eb[:, 0:nbank * 512],
                                             pr[:, 0:nbank * 512],
                                             AF.Exp, scale=0.125)
                        if dbg and j == NT - 1 and slot == "C" and g0 == 0:
                            nc.sync.dma_start(dbg_out["d_eb"][:, 0:nbank * 512],
                                              eb[:, 0:nbank * 512].bitcast(f32))
                        for b, ki, h in banks:
                            if ki >= 4 * j:  # diagonal band: causal mask
                                bsl = bass.ts(b, 512)
                                r = ki - 4 * j
                                nc.vector.tensor_mul(
                                    eb[:, bsl], eb[:, bsl],
                                    cmask[:, bass.ts(r, 512)])
                        for b, ki, h in banks:
                            nc.tensor.matmul(
                                att[h][:], v_aug[:, ki * 195 + 65 * h:
                                                 ki * 195 + 65 * h + 65],
                                eb[:, bass.ts(b, 512)],
                                start=(ki == 0), stop=(ki == nk - 1),
                                skip_group_check=True)
                    for h in heads:
                        if dbg and j == NT - 1 and h == 2:
                            datt = nrm.tile([65, 512], f32, tag="datt")
                            nc.vector.tensor_copy(datt[:], att[h][:])
                            nc.sync.dma_start(dbg_out["d_attv"], datt[:])
                        # denominator row (psum p64) -> sbuf, then broadcast
                        # across 64 partitions via a DRAM round-trip (stride-0
                        # leading dim is DRAM-only). Tile does not dep-track
                        # DRAM, so wire the RAW edge explicitly. The recip runs
                        # after the broadcast: custom-dve ops misbehave at
                        # nonzero base partitions.
                        scrA = nrm.tile([65, 512], f32, tag="scrA")
                        nc.vector.tensor_copy(scrA[64:65, :], att[h][64:65, :])
                        row_d = rsc_d[j * HPC + h, :]
                        wr = nc.sync.dma_start(row_d[None, :], scrA[64:65, :])
                        bc = nrm.tile([64, 512], f32, tag="bc")
                        rd = nc.gpsimd.dma_start(
                            out=bc[:], in_=bass.AP(row_d.tensor, row_d.offset,
                                                   [[0, 64], [1, 512]]))
                        add_dep_helper(rd.ins, wr.ins,
                                       reason="rscratch RAW (dram roundtrip)")
                        rcp = nrm.tile([64, 512], f32, tag="rcp")
                        nc.vector.reciprocal_approx_fast(out=rcp[:], in_=bc[:])
                        nc.vector.tensor_mul(aoT[h][:, jsl], att[h][0:64, :], rcp[:])
                        if dbg and j == NT - 1 and h == 2:
                            nc.sync.dma_start(dbg_out["d_bc"], bc[:])

        if dbg:
            for h in range(HPC):
                nc.sync.dma_start(dbg_out[f"d_ao{h}"], aoT[h][:].bitcast(f32))

        # ---------------- phase 4: output projection -------------------------
        with tc.tile_pool(name="pps", bufs=2, space="PSUM") as pr_ps, \
             tc.tile_pool(name="wpp", bufs=1) as wpp, \
             tc.tile_pool(name="yp", bufs=3) as y_pool:
            wp_sb = [wpp.tile([64, C], f32r, tag=f"wp{h}", name=f"wp{h}")
                     for h in range(HPC)]
            for h in range(HPC):
                nc.sync.dma_start(wp_sb[h][:], wp_d[h * 64:(h + 1) * 64, :])
            for m in range(T // 128 if kphase >= 4 else 0):
                msl = bass.ts(m, 128)
                y_sb = y_pool.tile([128, C], f32, tag="y")
                for ns in range(2):
                    py = pr_ps.tile([128, 384], f32, tag=f"py{ns}")
                    for h in range(HPC):
                        nc.tensor.matmul(py[:], aoT[h][:, msl],
                                         wp_sb[h][:, ns * 384:(ns + 1) * 384],
                                         start=(h == 0), stop=(h == HPC - 1))
                    nc.vector.tensor_copy(y_sb[:, ns * 384:(ns + 1) * 384], py[:])
                nc.sync.dma_start(y_d[m * 128:(m + 1) * 128, :], y_sb[:])

    nc.compile()
    return nc


_NC_CACHE = {}


def _get_nc(T):
    if T not in _NC_CACHE:
        _NC_CACHE[T] = build_nc(T)
    return _NC_CACHE[T]


def make_core_inputs(x, W_attn, b_attn, W_proj):
    """Host-side prep: per-core input dicts (see module docstring)."""
    B, T, _ = x.shape
    xts = [np.ascontiguousarray(x[b].T) for b in range(B)]
    in_maps = []
    for core in range(N_CORES):
        b = core // (N_CORES // B)
        h0 = HPC * (core % (N_CORES // B))
        ccols = slice(h0 * D, (h0 + 2) * D)      # first two heads
        c2 = slice((h0 + 2) * D, (h0 + 3) * D)   # third head
        # reference splits qkv as (k, q, v): k cols 0:C, q cols C:2C, v 2C:3C
        q01 = W_attn[:, C:2 * C][:, ccols]
        k01 = W_attn[:, 0:C][:, ccols]
        v01 = W_attn[:, 2 * C:3 * C][:, ccols]
        q2 = W_attn[:, C:2 * C][:, c2]
        k2 = W_attn[:, 0:C][:, c2]
        v2 = W_attn[:, 2 * C:3 * C][:, c2]
        wq = np.ascontiguousarray(
            np.concatenate([q01, k01, v01, q2, k2, v2], axis=1))
        bq = np.zeros((128, 5), np.float32)
        bq[:, 0] = b_attn[C:2 * C][ccols]
        bq[:, 1] = b_attn[0:C][ccols]
        bq[:, 2] = b_attn[2 * C:3 * C][ccols]
        bq[0:64, 3] = b_attn[C:2 * C][c2]
        bq[64:128, 3] = b_attn[0:C][c2]
        bq[0:64, 4] = b_attn[2 * C:3 * C][c2]
        wp = np.ascontiguousarray(W_proj[h0 * D:(h0 + HPC) * D, :])
        in_maps.append({"xt": xts[b], "wq": wq, "bq": bq, "wp": wp})
    return in_maps


def kernel(x, W_attn, b_attn, W_proj, b_proj):
    x = np.asarray(x, dtype=np.float32)
    W_attn = np.asarray(W_attn, dtype=np.float32)
    b_attn = np.asarray(b_attn, dtype=np.float32)
    W_proj = np.asarray(W_proj, dtype=np.float32)
    b_proj = np.asarray(b_proj, dtype=np.float32)
    B, T, _ = x.shape

    nc = _get_nc(T)
    in_maps = make_core_inputs(x, W_attn, b_attn, W_proj)
    res = None
    for attempt in range(3):
        try:
            res = run_bass_kernel_spmd(nc, in_maps, list(range(N_CORES)))
            break
        except Exception:
            # transient NRT_EXEC_UNIT_UNRECOVERABLE has been observed once
            # after a prior crashed process; a retry succeeds
            if attempt == 2:
                raise
    global LAST_RUN
    LAST_RUN = res

    gpb = N_CORES // B
    out = np.empty((B, T, C), np.float32)
    for b in range(B):
        acc = res.results[b * gpb]["y"].astype(np.float32)
        for g in range(1, gpb):
            acc = acc + res.results[b * gpb + g]["y"]
        out[b] = acc + b_proj[None, :]
    return out


# revision 25
# speedup vs baseline: 1.3949x; 1.0558x over previous
"""Causal self-attention (B=2, T=4096, C=768, H=12) on 8 TRN2 NeuronCores.

Sharding: batch x head-group. Core c handles batch b=c//4 and heads
h0..h0+2 where h0 = 3*(c%4). Each core computes qkv projection for its 3
heads, full causal attention, and a partial output projection; the host
sums the 4 partials per batch and adds the projection bias.

On-chip layout is feature-major ("transposed"): qT/kT [D, T] feed the
scores matmul directly, scores^T [k, q] feeds att@v with v in natural
layout, and the attention output stays transposed to feed the output
projection as the stationary operand (producing natural-layout y).
Matmuls run in float32r (~tf32). The softmax denominator comes free as a
65th "ones" column of v; normalization uses reciprocal_approx_fast + a
gpsimd partition broadcast.
"""

import sys

for _p in ("/opt/trn_rl_repo",):
    if _p not in sys.path:
        sys.path.insert(0, _p)

from contextlib import ExitStack

import numpy as np

import concourse.bass as bass  # noqa: F401  (engine classes referenced via nc)
import concourse.mybir as mybir
import concourse.tile as tile
from concourse import bacc
from concourse.bass_utils import run_bass_kernel_spmd
from concourse.masks import make_identity
from concourse.tile_rust import add_dep_helper

f32 = mybir.dt.float32
f32r = mybir.dt.float32r
AF = mybir.ActivationFunctionType

C = 768
D = 64
N_HEAD = 12
HPC = 3  # heads per core
N_CORES = 8

# wq column slots: q01 | k01 | v01 | (q2 stacked over k2) | v2
SLOTS = [(0, 128), (128, 256), (256, 384), (384, 512), (512, 576)]


def build_nc(T):
    NT = T // 512  # q tiles
    KT = T // 128  # k tiles
    CK = C // 128  # contraction chunks for qkv

    nc = bacc.Bacc("TRN2", target_bir_lowering=False, debug=False,
                   num_devices=N_CORES)
    xt_d = nc.dram_tensor("xt", [C, T], f32r, kind="ExternalInput").ap()
    wq_d = nc.dram_tensor("wq", [C, 576], f32r, kind="ExternalInput").ap()
    bq_d = nc.dram_tensor("bq", [128, 5], f32, kind="ExternalInput").ap()
    wp_d = nc.dram_tensor("wp", [HPC * D, C], f32r, kind="ExternalInput").ap()
    y_d = nc.dram_tensor("y", [T, C], f32, kind="ExternalOutput").ap()
    import os
    dbg = os.environ.get("KDBG") == "1"
    kphase = int(os.environ.get("KPHASE", "4"))
    # internal DRAM scratch for the softmax-reciprocal row broadcast
    rsc_d = nc.dram_tensor("rscratch", [NT * HPC, 512], f32,
                           **({"kind": "ExternalOutput"} if dbg else {})).ap()
    dbg_out = {}
    if dbg:
        for nm, shp in [("d_qAB", [128, T]), ("d_kAB", [128, T]),
                        ("d_qC", [128, T]), ("d_kC", [128, T]),
                        ("d_vaug", [128, KT * 195]),
                        ("d_ao0", [64, T]), ("d_ao1", [64, T]),
                        ("d_ao2", [64, T]), ("d_bc", [64, 512]),
                        ("d_eb", [128, 3072]), ("d_attv", [65, 512])]:
            dbg_out[nm] = nc.dram_tensor(nm, shp, f32, kind="ExternalOutput").ap()

    with tile.TileContext(nc) as tc, ExitStack() as ctx:
        sb = ctx.enter_context(tc.tile_pool(name="sb", bufs=1))

        # persistent tensors (live for the whole kernel)
        bq_sb = sb.tile([128, 5], f32, tag="bq")
        qT_AB = sb.tile([128, T], f32r, tag="qAB")
        kT_AB = sb.tile([128, T], f32r, tag="kAB")
        qT_C = sb.tile([128, T], f32r, tag="qC")
        kT_C = sb.tile([128, T], f32r, tag="kC")
        ident = sb.tile([128, 128], f32, tag="ident")
        ones_f = sb.tile([128, 1], f32, tag="ones")

        nc.sync.dma_start(bq_sb[:], bq_d)
        make_identity(nc, ident[:])
        nc.vector.memset(ones_f[:], 1.0)
        # causal masks for the 4 diagonal-band positions: keep col-p >= 128*r
        cmask = sb.tile([128, 4 * 512], f32, tag="cmask")
        nc.gpsimd.memset(cmask[:], 1.0)
        for r in range(4):
            nc.gpsimd.affine_select(
                cmask[:, r * 512:(r + 1) * 512], cmask[:, r * 512:(r + 1) * 512],
                pattern=[[1, 512]], compare_op=mybir.AluOpType.is_ge, fill=0.0,
                base=-128 * r, channel_multiplier=-1)

        # vaug lives phases 2-3; vpool (inside it) only phases 1-2
        vaugp = ctx.enter_context(tc.tile_pool(name="vaugp", bufs=1))
        es_v = ExitStack()
        vp = es_v.enter_context(tc.tile_pool(name="vpool", bufs=1))
        vT01 = vp.tile([128, T], f32, tag="v01")
        vT2 = vp.tile([64, T], f32, tag="v2")

        # ---------------- phase 1: qkv projection (transposed) --------------
        with tc.tile_pool(name="wqp", bufs=1) as wqp, \
             tc.tile_pool(name="xtp", bufs=3) as xt_pool, \
             tc.tile_pool(name="qkvps", bufs=1, space="PSUM") as qkv_ps:
            wq_sb = [wqp.tile([128, 576], f32r, tag=f"wq{c}", name=f"wq{c}")
                     for c in range(CK)]
            for c in range(CK):
                nc.sync.dma_start(wq_sb[c][:], wq_d[c * 128:(c + 1) * 128, :])
            for j in range(NT):
                jsl = bass.ts(j, 512)
                ps = [qkv_ps.tile([128, 512], f32, tag=f"s{k}", name=f"ps{k}") for k in range(4)]
                ps.append(qkv_ps.tile([64, 512], f32, tag="s4", name="ps4"))
                for c in range(CK):
                    xt_t = xt_pool.tile([128, 512], f32r, tag="xt")
                    nc.sync.dma_start(
                        xt_t[:], xt_d[c * 128:(c + 1) * 128, j * 512:(j + 1) * 512])
                    for s, (c0, c1) in enumerate(SLOTS):
                        nc.tensor.matmul(ps[s][:], wq_sb[c][:, c0:c1], xt_t[:],
                                         start=(c == 0), stop=(c == CK - 1))
                nc.vector.tensor_scalar_add(qT_AB[:, jsl], ps[0][:], bq_sb[:, 0:1])
                nc.vector.tensor_scalar_add(kT_AB[:, jsl], ps[1][:], bq_sb[:, 1:2])
                nc.vector.tensor_scalar_add(vT01[:, jsl], ps[2][:], bq_sb[:, 2:3])
                nc.vector.tensor_scalar_add(qT_C[0:64, jsl], ps[3][0:64, :],
                                            bq_sb[0:64, 3:4])
                nc.vector.tensor_scalar_add(kT_C[64:128, jsl], ps[3][64:128, :],
                                            bq_sb[64:128, 3:4])
                nc.vector.tensor_scalar_add(vT2[:, jsl], ps[4][:], bq_sb[0:64, 4:5])
            # duplicate head-2 q/k into the other 64-partition strip
            nc.sync.dma_start(qT_C[64:128, :], qT_C[0:64, :])
            nc.sync.dma_start(kT_C[0:64, :], kT_C[64:128, :])
            if dbg:
                nc.sync.dma_start(dbg_out["d_qAB"], qT_AB[:].bitcast(f32))
                nc.sync.dma_start(dbg_out["d_kAB"], kT_AB[:].bitcast(f32))
                nc.sync.dma_start(dbg_out["d_qC"], qT_C[:].bitcast(f32))
                nc.sync.dma_start(dbg_out["d_kC"], kT_C[:].bitcast(f32))

        # ---------------- phase 2: v -> natural layout + ones column --------
        if kphase >= 2:
          v_aug = vaugp.tile([128, KT * 195], f32r, tag="vaug")
          with tc.tile_pool(name="tps", bufs=3, space="PSUM") as tp_ps:
            for ki in range(KT):
                ksl = bass.ts(ki, 128)
                base = ki * 195
                p01 = tp_ps.tile([128, 128], f32, tag="tp01")
                nc.tensor.transpose(p01[:], vT01[:, ksl], ident[:])
                p2t = tp_ps.tile([128, 64], f32, tag="tp2")
                nc.tensor.transpose(p2t[:], vT2[:, ksl], ident[0:64, 0:64])
                nc.vector.tensor_copy(v_aug[:, base:base + 64], p01[:, 0:64])
                nc.vector.tensor_copy(v_aug[:, base + 65:base + 129], p01[:, 64:128])
                nc.vector.tensor_copy(v_aug[:, base + 130:base + 194], p2t[:])
            ones_cols = v_aug[:].rearrange("p (k c) -> p k c", c=65)[:, :, 64:65]
            nc.vector.tensor_copy(
                ones_cols, ones_f[:, 0:1, None].broadcast_to([128, 3 * KT, 1]))
          if dbg:
              nc.sync.dma_start(dbg_out["d_vaug"], v_aug[:].bitcast(f32))
          es_v.close()  # vT buffers no longer needed

          # ---------------- phase 3: attention -------------------------------
          aop = ctx.enter_context(tc.tile_pool(name="aop", bufs=1))
          aoT = [aop.tile([64, T], f32r, tag=f"aoT{h}", name=f"aoT{h}")
                 for h in range(HPC)]
          with tc.tile_pool(name="scps", bufs=2, space="PSUM") as sc_ps, \
             tc.tile_pool(name="avps", bufs=3, space="PSUM") as av_ps, \
             tc.tile_pool(name="pps", bufs=1, space="PSUM") as pr_ps, \
             tc.tile_pool(name="ebp", bufs=3) as eb_pool, \
             tc.tile_pool(name="wpp", bufs=1) as wpp, \
             tc.tile_pool(name="yp", bufs=3) as y_pool, \
             tc.tile_pool(name="nrm", bufs=3) as nrm:
            wp_sb = [wpp.tile([64, C], f32r, tag=f"wp{h}", name=f"wp{h}")
                     for h in range(HPC)]
            for h in range(HPC):
                nc.sync.dma_start(wp_sb[h][:], wp_d[h * 64:(h + 1) * 64, :])

            def emit_proj(m):
                msl = bass.ts(m, 128)
                y_sb = y_pool.tile([128, C], f32, tag="y", name="ysb")
                for ns in range(2):
                    py = pr_ps.tile([128, 384], f32, tag="py", name="py")
                    for h in range(HPC):
                        nc.tensor.matmul(py[:], aoT[h][:, msl],
                                         wp_sb[h][:, ns * 384:(ns + 1) * 384],
                                         start=(h == 0), stop=(h == HPC - 1))
                    nc.vector.tensor_copy(y_sb[:, ns * 384:(ns + 1) * 384],
                                          py[:])
                nc.sync.dma_start(y_d[m * 128:(m + 1) * 128, :], y_sb[:])

            for j in range(NT if kphase >= 3 else 0):
                jsl = bass.ts(j, 512)
                nk = 4 * j + 4
                for slot in ("AB", "C"):
                    if slot == "AB":
                        heads = [0, 1]
                        group = 1  # k-tiles per round (2 banks each)
                    else:
                        heads = [2]
                        group = 2
                    att = {h: av_ps.tile([65, 512], f32, tag="attv", name=f"attv{h}")
                           for h in heads}
                    for g0 in range(0, nk, group):
                        ks = list(range(g0, min(g0 + group, nk)))
                        nbank = len(ks) * len(heads)
                        pr = sc_ps.tile([128, 1024], f32, tag="sc")
                        banks = []  # (bank, ki, head)
                        for idx, ki in enumerate(ks):
                            ksl = bass.ts(ki, 128)
                            if slot == "AB":
                                for hh in (0, 1):
                                    b = idx * 2 + hh
                                    r0, r1 = 64 * hh, 64 * hh + 64
                                    nc.tensor.matmul(
                                        pr[:, bass.ts(b, 512)],
                                        kT_AB[r0:r1, ksl], qT_AB[r0:r1, jsl],
                                        start=True, stop=True)
                                    banks.append((b, ki, hh))
                            else:
                                strip = idx % 2
                                r0, r1 = 64 * strip, 64 * strip + 64
                                nc.tensor.matmul(
                                    pr[:, bass.ts(idx, 512)],
                                    kT_C[r0:r1, ksl], qT_C[r0:r1, jsl],
                                    start=True, stop=True)
                                banks.append((idx, ki, 2))
                        eb = eb_pool.tile([128, 1024], f32r, tag="eb")
                        nc.scalar.activation(eb[:, 0:nbank * 512],
                                             pr[:, 0:nbank * 512],
                                             AF.Exp, scale=0.125)
                        if dbg and j == NT - 1 and slot == "C" and g0 == 0:
                            nc.sync.dma_start(dbg_out["d_eb"][:, 0:nbank * 512],
                                              eb[:, 0:nbank * 512].bitcast(f32))
                        for b, ki, h in banks:
                            if ki >= 4 * j:  # diagonal band: causal mask
                                bsl = bass.ts(b, 512)
                                r = ki - 4 * j
                                nc.vector.tensor_mul(
                                    eb[:, bsl], eb[:, bsl],
                                    cmask[:, bass.ts(r, 512)])
                        for b, ki, h in banks:
                            nc.tensor.matmul(
                                att[h][:], v_aug[:, ki * 195 + 65 * h:
                                                 ki * 195 + 65 * h + 65],
                                eb[:, bass.ts(b, 512)],
                                start=(ki == 0), stop=(ki == nk - 1),
                                skip_group_check=True)
                    for h in heads:
                        if dbg and j == NT - 1 and h == 2:
                            datt = nrm.tile([65, 512], f32, tag="datt")
                            nc.vector.tensor_copy(datt[:], att[h][:])
                            nc.sync.dma_start(dbg_out["d_attv"], datt[:])
                        # denominator row (psum p64) -> sbuf, then broadcast
                        # across 64 partitions via a DRAM round-trip (stride-0
                        # leading dim is DRAM-only). Tile does not dep-track
                        # DRAM, so wire the RAW edge explicitly. The recip runs
                        # after the broadcast: custom-dve ops misbehave at
                        # nonzero base partitions.
                        scrA = nrm.tile([65, 512], f32, tag="scrA")
                        nc.vector.tensor_copy(scrA[64:65, :], att[h][64:65, :])
                        row_d = rsc_d[j * HPC + h, :]
                        wr = nc.sync.dma_start(row_d[None, :], scrA[64:65, :])
                        bc = nrm.tile([64, 512], f32, tag="bc")
                        rd = nc.gpsimd.dma_start(
                            out=bc[:], in_=bass.AP(row_d.tensor, row_d.offset,
                                                   [[0, 64], [1, 512]]))
                        add_dep_helper(rd.ins, wr.ins,
                                       reason="rscratch RAW (dram roundtrip)")
                        rcp = nrm.tile([64, 512], f32, tag="rcp")
                        nc.vector.reciprocal_approx_fast(out=rcp[:], in_=bc[:])
                        nc.vector.tensor_mul(aoT[h][:, jsl], att[h][0:64, :], rcp[:])
                        if dbg and j == NT - 1 and h == 2:
                            nc.sync.dma_start(dbg_out["d_bc"], bc[:])
                for m in range(4 * j, 4 * j + 4):
                    if kphase >= 4:
                        emit_proj(m)

        if dbg:
            for h in range(HPC):
                nc.sync.dma_start(dbg_out[f"d_ao{h}"], aoT[h][:].bitcast(f32))

    nc.compile()
    return nc


_NC_CACHE = {}


def _get_nc(T):
    if T not in _NC_CACHE:
        _NC_CACHE[T] = build_nc(T)
    return _NC_CACHE[T]


def make_core_inputs(x, W_attn, b_attn, W_proj):
    """Host-side prep: per-core input dicts (see module docstring)."""
    B, T, _ = x.shape
    xts = [np.ascontiguousarray(x[b].T) for b in range(B)]
    in_maps = []
    for core in range(N_CORES):
        b = core // (N_CORES // B)
        h0 = HPC * (core % (N_CORES // B))
        ccols = slice(h0 * D, (h0 + 2) * D)      # first two heads
        c2 = slice((h0 + 2) * D, (h0 + 3) * D)   # third head
        # reference splits qkv as (k, q, v): k cols 0:C, q cols C:2C, v 2C:3C
        q01 = W_attn[:, C:2 * C][:, ccols]
        k01 = W_attn[:, 0:C][:, ccols]
        v01 = W_attn[:, 2 * C:3 * C][:, ccols]
        q2 = W_attn[:, C:2 * C][:, c2]
        k2 = W_attn[:, 0:C][:, c2]
        v2 = W_attn[:, 2 * C:3 * C][:, c2]
        wq = np.ascontiguousarray(
            np.concatenate([q01, k01, v01, q2, k2, v2], axis=1))
        bq = np.zeros((128, 5), np.float32)
        bq[:, 0] = b_attn[C:2 * C][ccols]
        bq[:, 1] = b_attn[0:C][ccols]
        bq[:, 2] = b_attn[2 * C:3 * C][ccols]
        bq[0:64, 3] = b_attn[C:2 * C][c2]
        bq[64:128, 3] = b_attn[0:C][c2]
        bq[0:64, 4] = b_attn[2 * C:3 * C][c2]
        wp = np.ascontiguousarray(W_proj[h0 * D:(h0 + HPC) * D, :])
        in_maps.append({"xt": xts[b], "wq": wq, "bq": bq, "wp": wp})
    return in_maps


def kernel(x, W_attn, b_attn, W_proj, b_proj):
    x = np.asarray(x, dtype=np.float32)
    W_attn = np.asarray(W_attn, dtype=np.float32)
    b_attn = np.asarray(b_attn, dtype=np.float32)
    W_proj = np.asarray(W_proj, dtype=np.float32)
    b_proj = np.asarray(b_proj, dtype=np.float32)
    B, T, _ = x.shape

    nc = _get_nc(T)
    in_maps = make_core_inputs(x, W_attn, b_attn, W_proj)
    res = None
    for attempt in range(3):
        try:
            res = run_bass_kernel_spmd(nc, in_maps, list(range(N_CORES)))
            break
        except Exception:
            # transient NRT_EXEC_UNIT_UNRECOVERABLE has been observed once
            # after a prior crashed process; a retry succeeds
            if attempt == 2:
                raise
    global LAST_RUN
    LAST_RUN = res

    gpb = N_CORES // B
    out = np.empty((B, T, C), np.float32)
    for b in range(B):
        acc = res.results[b * gpb]["y"].astype(np.float32)
        for g in range(1, gpb):
            acc = acc + res.results[b * gpb + g]["y"]
        out[b] = acc + b_proj[None, :]
    return out


# revision 28
# speedup vs baseline: 1.4038x; 1.0064x over previous
"""Causal self-attention (B=2, T=4096, C=768, H=12) on 8 TRN2 NeuronCores.

Sharding: batch x head-group. Core c handles batch b=c//4 and heads
h0..h0+2 where h0 = 3*(c%4). Each core computes qkv projection for its 3
heads, full causal attention, and a partial output projection; the host
sums the 4 partials per batch and adds the projection bias.

On-chip layout is feature-major ("transposed"): qT/kT [D, T] feed the
scores matmul directly, scores^T [k, q] feeds att@v with v in natural
layout, and the attention output stays transposed to feed the output
projection as the stationary operand (producing natural-layout y).
Matmuls run in float32r (~tf32). The softmax denominator comes free as a
65th "ones" column of v; normalization uses reciprocal_approx_fast + a
gpsimd partition broadcast.
"""

import sys

for _p in ("/opt/trn_rl_repo",):
    if _p not in sys.path:
        sys.path.insert(0, _p)

from contextlib import ExitStack

import numpy as np

import concourse.bass as bass  # noqa: F401  (engine classes referenced via nc)
import concourse.mybir as mybir
import concourse.tile as tile
from concourse import bacc
from concourse.bass_utils import run_bass_kernel_spmd
from concourse.masks import make_identity
from concourse.tile_rust import add_dep_helper

f32 = mybir.dt.float32
f32r = mybir.dt.float32r
AF = mybir.ActivationFunctionType

C = 768
D = 64
N_HEAD = 12
HPC = 3  # heads per core
N_CORES = 8

# wq column slots: q01 | k01 | v01 | (q2 stacked over k2) | v2
SLOTS = [(0, 128), (128, 256), (256, 384), (384, 512), (512, 576)]


def build_nc(T):
    NT = T // 512  # q tiles
    KT = T // 128  # k tiles
    CK = C // 128  # contraction chunks for qkv

    nc = bacc.Bacc("TRN2", target_bir_lowering=False, debug=False,
                   num_devices=N_CORES)
    xt_d = nc.dram_tensor("xt", [C, T], f32r, kind="ExternalInput").ap()
    wq_d = nc.dram_tensor("wq", [C, 576], f32r, kind="ExternalInput").ap()
    bq_d = nc.dram_tensor("bq", [128, 5], f32, kind="ExternalInput").ap()
    wp_d = nc.dram_tensor("wp", [HPC * D, C], f32r, kind="ExternalInput").ap()
    y_d = nc.dram_tensor("y", [T, C], f32, kind="ExternalOutput").ap()
    import os
    dbg = os.environ.get("KDBG") == "1"
    kphase = int(os.environ.get("KPHASE", "4"))
    # internal DRAM scratch for the softmax-reciprocal row broadcast
    rsc_d = nc.dram_tensor("rscratch", [NT * HPC, 512], f32,
                           **({"kind": "ExternalOutput"} if dbg else {})).ap()
    dbg_out = {}
    if dbg:
        for nm, shp in [("d_qAB", [128, T]), ("d_kAB", [128, T]),
                        ("d_qC", [128, T]), ("d_kC", [128, T]),
                        ("d_vaug", [128, KT * 195]),
                        ("d_ao0", [64, T]), ("d_ao1", [64, T]),
                        ("d_ao2", [64, T]), ("d_bc", [64, 512]),
                        ("d_eb", [128, 3072]), ("d_attv", [65, 512])]:
            dbg_out[nm] = nc.dram_tensor(nm, shp, f32, kind="ExternalOutput").ap()

    with tile.TileContext(nc) as tc, ExitStack() as ctx:
        sb = ctx.enter_context(tc.tile_pool(name="sb", bufs=1))

        # persistent tensors (live for the whole kernel)
        bq_sb = sb.tile([128, 5], f32, tag="bq")
        qT_AB = sb.tile([128, T], f32r, tag="qAB")
        kT_AB = sb.tile([128, T], f32r, tag="kAB")
        qT_C = sb.tile([128, T], f32r, tag="qC")
        kT_C = sb.tile([128, T], f32r, tag="kC")
        ident = sb.tile([128, 128], f32, tag="ident")
        ones_f = sb.tile([128, 1], f32, tag="ones")

        nc.sync.dma_start(bq_sb[:], bq_d)
        make_identity(nc, ident[:])
        nc.vector.memset(ones_f[:], 1.0)
        # causal masks for the 4 diagonal-band positions: keep col-p >= 128*r
        cmask = sb.tile([128, 4 * 512], f32, tag="cmask")
        nc.gpsimd.memset(cmask[:], 1.0)
        for r in range(4):
            nc.gpsimd.affine_select(
                cmask[:, r * 512:(r + 1) * 512], cmask[:, r * 512:(r + 1) * 512],
                pattern=[[1, 512]], compare_op=mybir.AluOpType.is_ge, fill=0.0,
                base=-128 * r, channel_multiplier=-1)

        # vaug lives phases 2-3; vpool (inside it) only phases 1-2
        vaugp = ctx.enter_context(tc.tile_pool(name="vaugp", bufs=1))
        es_v = ExitStack()
        vp = es_v.enter_context(tc.tile_pool(name="vpool", bufs=1))
        vT01 = vp.tile([128, T], f32, tag="v01")
        vT2 = vp.tile([64, T], f32, tag="v2")

        # ---------------- phase 1: qkv projection (transposed) --------------
        with tc.tile_pool(name="wqp", bufs=1) as wqp, \
             tc.tile_pool(name="xtp", bufs=3) as xt_pool, \
             tc.tile_pool(name="qkvpsA", bufs=2, space="PSUM") as qkv_psA, \
             tc.tile_pool(name="qkvps", bufs=1, space="PSUM") as qkv_ps:
            wq_sb = [wqp.tile([128, 576], f32r, tag=f"wq{c}", name=f"wq{c}")
                     for c in range(CK)]
            for c in range(CK):
                nc.sync.dma_start(wq_sb[c][:], wq_d[c * 128:(c + 1) * 128, :])
            for j in range(NT):
                jsl = bass.ts(j, 512)
                ps = [qkv_psA.tile([128, 512], f32, tag=f"s{k}", name=f"ps{k}")
                      for k in range(3)]
                ps.append(qkv_ps.tile([128, 512], f32, tag="s3", name="ps3"))
                ps.append(qkv_ps.tile([64, 512], f32, tag="s4", name="ps4"))
                for c in range(CK):
                    xt_t = xt_pool.tile([128, 512], f32r, tag="xt")
                    nc.sync.dma_start(
                        xt_t[:], xt_d[c * 128:(c + 1) * 128, j * 512:(j + 1) * 512])
                    for s, (c0, c1) in enumerate(SLOTS):
                        nc.tensor.matmul(ps[s][:], wq_sb[c][:, c0:c1], xt_t[:],
                                         start=(c == 0), stop=(c == CK - 1))
                nc.vector.tensor_scalar_add(qT_AB[:, jsl], ps[0][:], bq_sb[:, 0:1])
                nc.vector.tensor_scalar_add(kT_AB[:, jsl], ps[1][:], bq_sb[:, 1:2])
                nc.vector.tensor_scalar_add(vT01[:, jsl], ps[2][:], bq_sb[:, 2:3])
                nc.vector.tensor_scalar_add(qT_C[0:64, jsl], ps[3][0:64, :],
                                            bq_sb[0:64, 3:4])
                nc.vector.tensor_scalar_add(kT_C[64:128, jsl], ps[3][64:128, :],
                                            bq_sb[64:128, 3:4])
                nc.vector.tensor_scalar_add(vT2[:, jsl], ps[4][:], bq_sb[0:64, 4:5])
            # duplicate head-2 q/k into the other 64-partition strip
            nc.sync.dma_start(qT_C[64:128, :], qT_C[0:64, :])
            nc.sync.dma_start(kT_C[0:64, :], kT_C[64:128, :])
            if dbg:
                nc.sync.dma_start(dbg_out["d_qAB"], qT_AB[:].bitcast(f32))
                nc.sync.dma_start(dbg_out["d_kAB"], kT_AB[:].bitcast(f32))
                nc.sync.dma_start(dbg_out["d_qC"], qT_C[:].bitcast(f32))
                nc.sync.dma_start(dbg_out["d_kC"], kT_C[:].bitcast(f32))

        # ---------------- phase 2: v -> natural layout + ones column --------
        if kphase >= 2:
          v_aug = vaugp.tile([128, KT * 195], f32r, tag="vaug")
          with tc.tile_pool(name="tps", bufs=3, space="PSUM") as tp_ps:
            for ki in range(KT):
                ksl = bass.ts(ki, 128)
                base = ki * 195
                p01 = tp_ps.tile([128, 128], f32, tag="tp01")
                nc.tensor.transpose(p01[:], vT01[:, ksl], ident[:])
                p2t = tp_ps.tile([128, 64], f32, tag="tp2")
                nc.tensor.transpose(p2t[:], vT2[:, ksl], ident[0:64, 0:64])
                nc.vector.tensor_copy(v_aug[:, base:base + 64], p01[:, 0:64])
                nc.vector.tensor_copy(v_aug[:, base + 65:base + 129], p01[:, 64:128])
                nc.vector.tensor_copy(v_aug[:, base + 130:base + 194], p2t[:])
            ones_cols = v_aug[:].rearrange("p (k c) -> p k c", c=65)[:, :, 64:65]
            nc.vector.tensor_copy(
                ones_cols, ones_f[:, 0:1, None].broadcast_to([128, 3 * KT, 1]))
          if dbg:
              nc.sync.dma_start(dbg_out["d_vaug"], v_aug[:].bitcast(f32))
          es_v.close()  # vT buffers no longer needed

          # ---------------- phase 3: attention -------------------------------
          aop = ctx.enter_context(tc.tile_pool(name="aop", bufs=1))
          aoT = [aop.tile([64, T], f32r, tag=f"aoT{h}", name=f"aoT{h}")
                 for h in range(HPC)]
          with tc.tile_pool(name="scps", bufs=2, space="PSUM") as sc_ps, \
             tc.tile_pool(name="avps", bufs=3, space="PSUM") as av_ps, \
             tc.tile_pool(name="pps", bufs=1, space="PSUM") as pr_ps, \
             tc.tile_pool(name="ebp", bufs=4) as eb_pool, \
             tc.tile_pool(name="wpp", bufs=1) as wpp, \
             tc.tile_pool(name="yp", bufs=3) as y_pool, \
             tc.tile_pool(name="nrm", bufs=3) as nrm:
            wp_sb = [wpp.tile([64, C], f32r, tag=f"wp{h}", name=f"wp{h}")
                     for h in range(HPC)]
            for h in range(HPC):
                nc.sync.dma_start(wp_sb[h][:], wp_d[h * 64:(h + 1) * 64, :])

            def emit_proj(m):
                msl = bass.ts(m, 128)
                y_sb = y_pool.tile([128, C], f32, tag="y", name="ysb")
                for ns in range(2):
                    py = pr_ps.tile([128, 384], f32, tag="py", name="py")
                    for h in range(HPC):
                        nc.tensor.matmul(py[:], aoT[h][:, msl],
                                         wp_sb[h][:, ns * 384:(ns + 1) * 384],
                                         start=(h == 0), stop=(h == HPC - 1))
                    nc.vector.tensor_copy(y_sb[:, ns * 384:(ns + 1) * 384],
                                          py[:])
                nc.sync.dma_start(y_d[m * 128:(m + 1) * 128, :], y_sb[:])

            for j in range(NT if kphase >= 3 else 0):
                jsl = bass.ts(j, 512)
                nk = 4 * j + 4
                for slot in ("AB", "C"):
                    if slot == "AB":
                        heads = [0, 1]
                        group = 1  # k-tiles per round (2 banks each)
                    else:
                        heads = [2]
                        group = 2
                    att = {h: av_ps.tile([65, 512], f32, tag="attv", name=f"attv{h}")
                           for h in heads}
                    for g0 in range(0, nk, group):
                        ks = list(range(g0, min(g0 + group, nk)))
                        nbank = len(ks) * len(heads)
                        pr = sc_ps.tile([128, 1024], f32, tag="sc")
                        banks = []  # (bank, ki, head)
                        for idx, ki in enumerate(ks):
                            ksl = bass.ts(ki, 128)
                            if slot == "AB":
                                for hh in (0, 1):
                                    b = idx * 2 + hh
                                    r0, r1 = 64 * hh, 64 * hh + 64
                                    nc.tensor.matmul(
                                        pr[:, bass.ts(b, 512)],
                                        kT_AB[r0:r1, ksl], qT_AB[r0:r1, jsl],
                                        start=True, stop=True)
                                    banks.append((b, ki, hh))
                            else:
                                strip = idx % 2
                                r0, r1 = 64 * strip, 64 * strip + 64
                                nc.tensor.matmul(
                                    pr[:, bass.ts(idx, 512)],
                                    kT_C[r0:r1, ksl], qT_C[r0:r1, jsl],
                                    start=True, stop=True)
                                banks.append((idx, ki, 2))
                        eb = eb_pool.tile([128, 1024], f32r, tag="eb")
                        nc.scalar.activation(eb[:, 0:nbank * 512],
                                             pr[:, 0:nbank * 512],
                                             AF.Exp, scale=0.125)
                        if dbg and j == NT - 1 and slot == "C" and g0 == 0:
                            nc.sync.dma_start(dbg_out["d_eb"][:, 0:nbank * 512],
                                              eb[:, 0:nbank * 512].bitcast(f32))
                        for b, ki, h in banks:
                            if ki >= 4 * j:  # diagonal band: causal mask
                                bsl = bass.ts(b, 512)
                                r = ki - 4 * j
                                nc.vector.tensor_mul(
                                    eb[:, bsl], eb[:, bsl],
                                    cmask[:, bass.ts(r, 512)])
                        for b, ki, h in banks:
                            nc.tensor.matmul(
                                att[h][:], v_aug[:, ki * 195 + 65 * h:
                                                 ki * 195 + 65 * h + 65],
                                eb[:, bass.ts(b, 512)],
                                start=(ki == 0), stop=(ki == nk - 1),
                                skip_group_check=True)
                    for h in heads:
                        if dbg and j == NT - 1 and h == 2:
                            datt = nrm.tile([65, 512], f32, tag="datt")
                            nc.vector.tensor_copy(datt[:], att[h][:])
                            nc.sync.dma_start(dbg_out["d_attv"], datt[:])
                        # denominator row (psum p64) -> sbuf, then broadcast
                        # across 64 partitions via a DRAM round-trip (stride-0
                        # leading dim is DRAM-only). Tile does not dep-track
                        # DRAM, so wire the RAW edge explicitly. The recip runs
                        # after the broadcast: custom-dve ops misbehave at
                        # nonzero base partitions.
                        scrA = nrm.tile([65, 512], f32, tag="scrA")
                        nc.vector.tensor_copy(scrA[64:65, :], att[h][64:65, :])
                        row_d = rsc_d[j * HPC + h, :]
                        wr = nc.sync.dma_start(row_d[None, :], scrA[64:65, :])
                        bc = nrm.tile([64, 512], f32, tag="bc")
                        rd = nc.gpsimd.dma_start(
                            out=bc[:], in_=bass.AP(row_d.tensor, row_d.offset,
                                                   [[0, 64], [1, 512]]))
                        add_dep_helper(rd.ins, wr.ins,
                                       reason="rscratch RAW (dram roundtrip)")
                        rcp = nrm.tile([64, 512], f32, tag="rcp")
                        nc.vector.reciprocal_approx_fast(out=rcp[:], in_=bc[:])
                        nc.vector.tensor_mul(aoT[h][:, jsl], att[h][0:64, :], rcp[:])
                        if dbg and j == NT - 1 and h == 2:
                            nc.sync.dma_start(dbg_out["d_bc"], bc[:])
                for m in range(4 * j, 4 * j + 4):
                    if kphase >= 4:
                        emit_proj(m)

        if dbg:
            for h in range(HPC):
                nc.sync.dma_start(dbg_out[f"d_ao{h}"], aoT[h][:].bitcast(f32))

    nc.compile()
    return nc


_NC_CACHE = {}


def _get_nc(T):
    if T not in _NC_CACHE:
        _NC_CACHE[T] = build_nc(T)
    return _NC_CACHE[T]


def make_core_inputs(x, W_attn, b_attn, W_proj):
    """Host-side prep: per-core input dicts (see module docstring)."""
    B, T, _ = x.shape
    xts = [np.ascontiguousarray(x[b].T) for b in range(B)]
    in_maps = []
    for core in range(N_CORES):
        b = core // (N_CORES // B)
        h0 = HPC * (core % (N_CORES // B))
        ccols = slice(h0 * D, (h0 + 2) * D)      # first two heads
        c2 = slice((h0 + 2) * D, (h0 + 3) * D)   # third head
        # reference splits qkv as (k, q, v): k cols 0:C, q cols C:2C, v 2C:3C
        q01 = W_attn[:, C:2 * C][:, ccols]
        k01 = W_attn[:, 0:C][:, ccols]
        v01 = W_attn[:, 2 * C:3 * C][:, ccols]
        q2 = W_attn[:, C:2 * C][:, c2]
        k2 = W_attn[:, 0:C][:, c2]
        v2 = W_attn[:, 2 * C:3 * C][:, c2]
        wq = np.ascontiguousarray(
            np.concatenate([q01, k01, v01, q2, k2, v2], axis=1))
        bq = np.zeros((128, 5), np.float32)
        bq[:, 0] = b_attn[C:2 * C][ccols]
        bq[:, 1] = b_attn[0:C][ccols]
        bq[:, 2] = b_attn[2 * C:3 * C][ccols]
        bq[0:64, 3] = b_attn[C:2 * C][c2]
        bq[64:128, 3] = b_attn[0:C][c2]
        bq[0:64, 4] = b_attn[2 * C:3 * C][c2]
        wp = np.ascontiguousarray(W_proj[h0 * D:(h0 + HPC) * D, :])
        in_maps.append({"xt": xts[b], "wq": wq, "bq": bq, "wp": wp})
    return in_maps


def kernel(x, W_attn, b_attn, W_proj, b_proj):
    x = np.asarray(x, dtype=np.float32)
    W_attn = np.asarray(W_attn, dtype=np.float32)
    b_attn = np.asarray(b_attn, dtype=np.float32)
    W_proj = np.asarray(W_proj, dtype=np.float32)
    b_proj = np.asarray(b_proj, dtype=np.float32)
    B, T, _ = x.shape

    nc = _get_nc(T)
    in_maps = make_core_inputs(x, W_attn, b_attn, W_proj)
    res = None
    for attempt in range(3):
        try:
            res = run_bass_kernel_spmd(nc, in_maps, list(range(N_CORES)))
            break
        except Exception:
            # transient NRT_EXEC_UNIT_UNRECOVERABLE has been observed once
            # after a prior crashed process; a retry succeeds
            if attempt == 2:
                raise
    global LAST_RUN
    LAST_RUN = res

    gpb = N_CORES // B
    out = np.empty((B, T, C), np.float32)
    for b in range(B):
        acc = res.results[b * gpb]["y"].astype(np.float32)
        for g in range(1, gpb):
            acc = acc + res.results[b * gpb + g]["y"]
        out[b] = acc + b_proj[None, :]
    return out


# revision 32
# speedup vs baseline: 1.5048x; 1.0719x over previous
"""Causal self-attention (B=2, T=4096, C=768, H=12) on 8 TRN2 NeuronCores.

Sharding: batch x head-group. Core c handles batch b=c//4 and heads
h0..h0+2 where h0 = 3*(c%4). Each core computes qkv projection for its 3
heads, full causal attention, and a partial output projection; the host
sums the 4 partials per batch and adds the projection bias.

On-chip layout is feature-major ("transposed"): qT/kT [D, T] feed the
scores matmul directly, scores^T [k, q] feeds att@v with v in natural
layout, and the attention output stays transposed to feed the output
projection as the stationary operand (producing natural-layout y).
Matmuls run in float32r (~tf32). The softmax denominator comes free as a
65th "ones" column of v; normalization uses reciprocal_approx_fast + a
gpsimd partition broadcast.
"""

import sys

for _p in ("/opt/trn_rl_repo",):
    if _p not in sys.path:
        sys.path.insert(0, _p)

from contextlib import ExitStack

import numpy as np

import concourse.bass as bass  # noqa: F401  (engine classes referenced via nc)
import concourse.mybir as mybir
import concourse.tile as tile
from concourse import bacc
from concourse.bass_utils import run_bass_kernel_spmd
from concourse.masks import make_identity
from concourse.tile_rust import add_dep_helper

f32 = mybir.dt.float32
f32r = mybir.dt.float32r
AF = mybir.ActivationFunctionType

C = 768
D = 64
N_HEAD = 12
HPC = 3  # heads per core
N_CORES = 8

# wq column slots: q01 | k01 | v01 | (q2 stacked over k2) | v2
SLOTS = [(0, 128), (128, 256), (256, 384), (384, 512), (512, 576)]


def build_nc(T):
    NT = T // 512  # q tiles
    KT = T // 128  # k tiles
    CK = C // 128  # contraction chunks for qkv

    nc = bacc.Bacc("TRN2", target_bir_lowering=False, debug=False,
                   num_devices=N_CORES)
    xt_d = nc.dram_tensor("xt", [C, T], f32r, kind="ExternalInput").ap()
    wq_d = nc.dram_tensor("wq", [C, 576], f32r, kind="ExternalInput").ap()
    bq_d = nc.dram_tensor("bq", [128, 5], f32, kind="ExternalInput").ap()
    wp_d = nc.dram_tensor("wp", [HPC * D, C], f32r, kind="ExternalInput").ap()
    y_d = nc.dram_tensor("y", [T, C], f32, kind="ExternalOutput").ap()
    import os
    dbg = os.environ.get("KDBG") == "1"
    kphase = int(os.environ.get("KPHASE", "4"))
    # internal DRAM scratch for the softmax-reciprocal row broadcast
    rsc_d = nc.dram_tensor("rscratch", [NT * HPC, 512], f32,
                           **({"kind": "ExternalOutput"} if dbg else {})).ap()
    dbg_out = {}
    if dbg:
        for nm, shp in [("d_qAB", [128, T]), ("d_kAB", [128, T]),
                        ("d_qC", [128, T]), ("d_kC", [128, T]),
                        ("d_vaug", [128, KT * 195]),
                        ("d_ao0", [64, T]), ("d_ao1", [64, T]),
                        ("d_ao2", [64, T]), ("d_bc", [64, 512]),
                        ("d_eb", [128, 3072]), ("d_attv", [65, 512])]:
            dbg_out[nm] = nc.dram_tensor(nm, shp, f32, kind="ExternalOutput").ap()

    with tile.TileContext(nc) as tc, ExitStack() as ctx:
        sb = ctx.enter_context(tc.tile_pool(name="sb", bufs=1))

        # persistent tensors (live for the whole kernel)
        bq_sb = sb.tile([128, 5], f32, tag="bq")
        qT_AB = sb.tile([128, T], f32r, tag="qAB")
        kT_AB = sb.tile([128, T], f32r, tag="kAB")
        qT_C = sb.tile([128, T], f32r, tag="qC")
        kT_C = sb.tile([128, T], f32r, tag="kC")
        ident = sb.tile([128, 128], f32, tag="ident")
        ones_f = sb.tile([128, 1], f32, tag="ones")

        nc.sync.dma_start(bq_sb[:], bq_d)
        make_identity(nc, ident[:])
        nc.vector.memset(ones_f[:], 1.0)
        # causal masks for the 4 diagonal-band positions: keep col-p >= 128*r
        cmask = sb.tile([128, 4 * 512], f32, tag="cmask")
        nc.gpsimd.memset(cmask[:], 1.0)
        for r in range(4):
            nc.gpsimd.affine_select(
                cmask[:, r * 512:(r + 1) * 512], cmask[:, r * 512:(r + 1) * 512],
                pattern=[[1, 512]], compare_op=mybir.AluOpType.is_ge, fill=0.0,
                base=-128 * r, channel_multiplier=-1)

        # vaug lives phases 2-3; vpool (inside it) only phases 1-2
        vaugp = ctx.enter_context(tc.tile_pool(name="vaugp", bufs=1))
        es_v = ExitStack()
        vp = es_v.enter_context(tc.tile_pool(name="vpool", bufs=1))
        vT01 = vp.tile([128, T], f32, tag="v01")
        vT2 = vp.tile([64, T], f32, tag="v2")

        # ---------------- phase 1: qkv projection (transposed) --------------
        with tc.tile_pool(name="wqp", bufs=1) as wqp, \
             tc.tile_pool(name="xtp", bufs=6) as xt_pool, \
             tc.tile_pool(name="qkvpsA", bufs=2, space="PSUM") as qkv_psA, \
             tc.tile_pool(name="qkvps", bufs=1, space="PSUM") as qkv_ps:
            wq_sb = [wqp.tile([128, 576], f32r, tag=f"wq{c}", name=f"wq{c}")
                     for c in range(CK)]
            for c in range(CK):
                nc.sync.dma_start(wq_sb[c][:], wq_d[c * 128:(c + 1) * 128, :])
            for j in range(NT):
                jsl = bass.ts(j, 512)
                ps = [qkv_psA.tile([128, 512], f32, tag=f"s{k}", name=f"ps{k}")
                      for k in range(3)]
                ps.append(qkv_ps.tile([128, 512], f32, tag="s3", name="ps3"))
                ps.append(qkv_ps.tile([64, 512], f32, tag="s4", name="ps4"))
                for c in range(CK):
                    xt_t = xt_pool.tile([128, 512], f32r, tag="xt")
                    nc.sync.dma_start(
                        xt_t[:], xt_d[c * 128:(c + 1) * 128, j * 512:(j + 1) * 512])
                    for s, (c0, c1) in enumerate(SLOTS):
                        nc.tensor.matmul(ps[s][:], wq_sb[c][:, c0:c1], xt_t[:],
                                         start=(c == 0), stop=(c == CK - 1))
                nc.vector.tensor_scalar_add(qT_AB[:, jsl], ps[0][:], bq_sb[:, 0:1])
                nc.vector.tensor_scalar_add(kT_AB[:, jsl], ps[1][:], bq_sb[:, 1:2])
                nc.vector.tensor_scalar_add(vT01[:, jsl], ps[2][:], bq_sb[:, 2:3])
                nc.vector.tensor_scalar_add(qT_C[0:64, jsl], ps[3][0:64, :],
                                            bq_sb[0:64, 3:4])
                nc.vector.tensor_scalar_add(kT_C[64:128, jsl], ps[3][64:128, :],
                                            bq_sb[64:128, 3:4])
                nc.vector.tensor_scalar_add(vT2[:, jsl], ps[4][:], bq_sb[0:64, 4:5])
            # duplicate head-2 q/k into the other 64-partition strip
            nc.sync.dma_start(qT_C[64:128, :], qT_C[0:64, :])
            nc.sync.dma_start(kT_C[0:64, :], kT_C[64:128, :])
            if dbg:
                nc.sync.dma_start(dbg_out["d_qAB"], qT_AB[:].bitcast(f32))
                nc.sync.dma_start(dbg_out["d_kAB"], kT_AB[:].bitcast(f32))
                nc.sync.dma_start(dbg_out["d_qC"], qT_C[:].bitcast(f32))
                nc.sync.dma_start(dbg_out["d_kC"], kT_C[:].bitcast(f32))

        # ---------------- phase 2: v -> natural layout + ones column --------
        if kphase >= 2:
          v_aug = vaugp.tile([128, KT * 195], f32r, tag="vaug")
          with tc.tile_pool(name="tps", bufs=3, space="PSUM") as tp_ps:
            for ki in range(KT):
                ksl = bass.ts(ki, 128)
                base = ki * 195
                p01 = tp_ps.tile([128, 128], f32, tag="tp01")
                nc.tensor.transpose(p01[:], vT01[:, ksl], ident[:])
                p2t = tp_ps.tile([128, 64], f32, tag="tp2")
                nc.tensor.transpose(p2t[:], vT2[:, ksl], ident[0:64, 0:64])
                nc.vector.tensor_copy(v_aug[:, base:base + 64], p01[:, 0:64])
                nc.vector.tensor_copy(v_aug[:, base + 65:base + 129], p01[:, 64:128])
                nc.vector.tensor_copy(v_aug[:, base + 130:base + 194], p2t[:])
            ones_cols = v_aug[:].rearrange("p (k c) -> p k c", c=65)[:, :, 64:65]
            nc.vector.tensor_copy(
                ones_cols, ones_f[:, 0:1, None].broadcast_to([128, 3 * KT, 1]))
          if dbg:
              nc.sync.dma_start(dbg_out["d_vaug"], v_aug[:].bitcast(f32))
          es_v.close()  # vT buffers no longer needed

          # ---------------- phase 3: attention -------------------------------
          aop = ctx.enter_context(tc.tile_pool(name="aop", bufs=1))
          aoT = [aop.tile([64, T], f32r, tag=f"aoT{h}", name=f"aoT{h}")
                 for h in range(HPC)]
          with tc.tile_pool(name="scps", bufs=2, space="PSUM") as sc_ps, \
             tc.tile_pool(name="avps", bufs=3, space="PSUM") as av_ps, \
             tc.tile_pool(name="pps", bufs=1, space="PSUM") as pr_ps, \
             tc.tile_pool(name="ebp", bufs=6) as eb_pool, \
             tc.tile_pool(name="wpp", bufs=1) as wpp, \
             tc.tile_pool(name="yp", bufs=3) as y_pool, \
             tc.tile_pool(name="nrm", bufs=3) as nrm:
            wp_sb = [wpp.tile([64, C], f32r, tag=f"wp{h}", name=f"wp{h}")
                     for h in range(HPC)]
            for h in range(HPC):
                nc.sync.dma_start(wp_sb[h][:], wp_d[h * 64:(h + 1) * 64, :])

            def emit_proj(m):
                msl = bass.ts(m, 128)
                y_sb = y_pool.tile([128, C], f32, tag="y", name="ysb")
                for ns in range(2):
                    py = pr_ps.tile([128, 384], f32, tag="py", name="py")
                    for h in range(HPC):
                        nc.tensor.matmul(py[:], aoT[h][:, msl],
                                         wp_sb[h][:, ns * 384:(ns + 1) * 384],
                                         start=(h == 0), stop=(h == HPC - 1))
                    nc.vector.tensor_copy(y_sb[:, ns * 384:(ns + 1) * 384],
                                          py[:])
                nc.sync.dma_start(y_d[m * 128:(m + 1) * 128, :], y_sb[:])

            for j in range(NT if kphase >= 3 else 0):
                jsl = bass.ts(j, 512)
                nk = 4 * j + 4
                for slot in ("AB", "C"):
                    if slot == "AB":
                        heads = [0, 1]
                        group = 1  # k-tiles per round (2 banks each)
                    else:
                        heads = [2]
                        group = 2
                    att = {h: av_ps.tile([65, 512], f32, tag="attv", name=f"attv{h}")
                           for h in heads}
                    for g0 in range(0, nk, group):
                        ks = list(range(g0, min(g0 + group, nk)))
                        nbank = len(ks) * len(heads)
                        pr = sc_ps.tile([128, 1024], f32, tag="sc")
                        banks = []  # (bank, ki, head)
                        for idx, ki in enumerate(ks):
                            ksl = bass.ts(ki, 128)
                            if slot == "AB":
                                for hh in (0, 1):
                                    b = idx * 2 + hh
                                    r0, r1 = 64 * hh, 64 * hh + 64
                                    nc.tensor.matmul(
                                        pr[:, bass.ts(b, 512)],
                                        kT_AB[r0:r1, ksl], qT_AB[r0:r1, jsl],
                                        start=True, stop=True)
                                    banks.append((b, ki, hh))
                            else:
                                strip = idx % 2
                                r0, r1 = 64 * strip, 64 * strip + 64
                                nc.tensor.matmul(
                                    pr[:, bass.ts(idx, 512)],
                                    kT_C[r0:r1, ksl], qT_C[r0:r1, jsl],
                                    start=True, stop=True)
                                banks.append((idx, ki, 2))
                        eb = eb_pool.tile([128, 1024], f32r, tag="eb")
                        nc.scalar.activation(eb[:, 0:nbank * 512],
                                             pr[:, 0:nbank * 512],
                                             AF.Exp, scale=0.125)
                        if dbg and j == NT - 1 and slot == "C" and g0 == 0:
                            nc.sync.dma_start(dbg_out["d_eb"][:, 0:nbank * 512],
                                              eb[:, 0:nbank * 512].bitcast(f32))
                        for b, ki, h in banks:
                            if ki >= 4 * j:  # diagonal band: causal mask
                                bsl = bass.ts(b, 512)
                                r = ki - 4 * j
                                nc.vector.tensor_mul(
                                    eb[:, bsl], eb[:, bsl],
                                    cmask[:, bass.ts(r, 512)])
                        for b, ki, h in banks:
                            nc.tensor.matmul(
                                att[h][:], v_aug[:, ki * 195 + 65 * h:
                                                 ki * 195 + 65 * h + 65],
                                eb[:, bass.ts(b, 512)],
                                start=(ki == 0), stop=(ki == nk - 1),
                                skip_group_check=True)
                    for h in heads:
                        if dbg and j == NT - 1 and h == 2:
                            datt = nrm.tile([65, 512], f32, tag="datt")
                            nc.vector.tensor_copy(datt[:], att[h][:])
                            nc.sync.dma_start(dbg_out["d_attv"], datt[:])
                        # denominator row (psum p64) -> sbuf, then broadcast
                        # across 64 partitions via a DRAM round-trip (stride-0
                        # leading dim is DRAM-only). Tile does not dep-track
                        # DRAM, so wire the RAW edge explicitly. The recip runs
                        # after the broadcast: custom-dve ops misbehave at
                        # nonzero base partitions.
                        scrA = nrm.tile([65, 512], f32, tag="scrA")
                        nc.vector.tensor_copy(scrA[64:65, :], att[h][64:65, :])
                        row_d = rsc_d[j * HPC + h, :]
                        wr = nc.sync.dma_start(row_d[None, :], scrA[64:65, :])
                        bc = nrm.tile([64, 512], f32, tag="bc")
                        rd = nc.gpsimd.dma_start(
                            out=bc[:], in_=bass.AP(row_d.tensor, row_d.offset,
                                                   [[0, 64], [1, 512]]))
                        add_dep_helper(rd.ins, wr.ins,
                                       reason="rscratch RAW (dram roundtrip)")
                        rcp = nrm.tile([64, 512], f32, tag="rcp")
                        nc.vector.reciprocal_approx_fast(out=rcp[:], in_=bc[:])
                        nc.vector.tensor_mul(aoT[h][:, jsl], att[h][0:64, :], rcp[:])
                        if dbg and j == NT - 1 and h == 2:
                            nc.sync.dma_start(dbg_out["d_bc"], bc[:])
                for m in range(4 * j, 4 * j + 4):
                    if kphase >= 4:
                        emit_proj(m)

        if dbg:
            for h in range(HPC):
                nc.sync.dma_start(dbg_out[f"d_ao{h}"], aoT[h][:].bitcast(f32))

    nc.compile()
    return nc


_NC_CACHE = {}


def _get_nc(T):
    if T not in _NC_CACHE:
        _NC_CACHE[T] = build_nc(T)
    return _NC_CACHE[T]


def make_core_inputs(x, W_attn, b_attn, W_proj):
    """Host-side prep: per-core input dicts (see module docstring)."""
    B, T, _ = x.shape
    xts = [np.ascontiguousarray(x[b].T) for b in range(B)]
    in_maps = []
    for core in range(N_CORES):
        b = core // (N_CORES // B)
        h0 = HPC * (core % (N_CORES // B))
        ccols = slice(h0 * D, (h0 + 2) * D)      # first two heads
        c2 = slice((h0 + 2) * D, (h0 + 3) * D)   # third head
        # reference splits qkv as (k, q, v): k cols 0:C, q cols C:2C, v 2C:3C
        q01 = W_attn[:, C:2 * C][:, ccols]
        k01 = W_attn[:, 0:C][:, ccols]
        v01 = W_attn[:, 2 * C:3 * C][:, ccols]
        q2 = W_attn[:, C:2 * C][:, c2]
        k2 = W_attn[:, 0:C][:, c2]
        v2 = W_attn[:, 2 * C:3 * C][:, c2]
        wq = np.ascontiguousarray(
            np.concatenate([q01, k01, v01, q2, k2, v2], axis=1))
        bq = np.zeros((128, 5), np.float32)
        bq[:, 0] = b_attn[C:2 * C][ccols]
        bq[:, 1] = b_attn[0:C][ccols]
        bq[:, 2] = b_attn[2 * C:3 * C][ccols]
        bq[0:64, 3] = b_attn[C:2 * C][c2]
        bq[64:128, 3] = b_attn[0:C][c2]
        bq[0:64, 4] = b_attn[2 * C:3 * C][c2]
        wp = np.ascontiguousarray(W_proj[h0 * D:(h0 + HPC) * D, :])
        in_maps.append({"xt": xts[b], "wq": wq, "bq": bq, "wp": wp})
    return in_maps


def kernel(x, W_attn, b_attn, W_proj, b_proj):
    x = np.asarray(x, dtype=np.float32)
    W_attn = np.asarray(W_attn, dtype=np.float32)
    b_attn = np.asarray(b_attn, dtype=np.float32)
    W_proj = np.asarray(W_proj, dtype=np.float32)
    b_proj = np.asarray(b_proj, dtype=np.float32)
    B, T, _ = x.shape

    nc = _get_nc(T)
    in_maps = make_core_inputs(x, W_attn, b_attn, W_proj)
    res = None
    for attempt in range(3):
        try:
            res = run_bass_kernel_spmd(nc, in_maps, list(range(N_CORES)))
            break
        except Exception:
            # transient NRT_EXEC_UNIT_UNRECOVERABLE has been observed once
            # after a prior crashed process; a retry succeeds
            if attempt == 2:
                raise
    global LAST_RUN
    LAST_RUN = res

    gpb = N_CORES // B
    out = np.empty((B, T, C), np.float32)
    for b in range(B):
        acc = res.results[b * gpb]["y"].astype(np.float32)
        for g in range(1, gpb):
            acc = acc + res.results[b * gpb + g]["y"]
        out[b] = acc + b_proj[None, :]
    return out
